# revision 3
# baseline (speedup 1.0000x reference)
"""CorrelationDimensionLoss kernel for 8x Trainium2 NeuronCores (Bass, raw engines).

Math: reference computes S_m = sum_{i<j} sigmoid(K*(r_m - d_ij)) for 16
log-spaced thresholds r_m, divides by pair count, then returns -slope of
lstsq(log r, log S).

Design (vs the 13-ACT-pass baseline at ~381us):
  - PE computes d^2 per 128x512 tile via one K=36 augmented fp16 matmul
    ([-2x, 1, 1, |x|2_hi, |x|2_lo] . [x, |x|2_hi, |x|2_lo, 1, 1]; split |x|^2
    keeps the d^2 encoding error at ~4e-3).
  - ACT drains PSUM with the sqrt: d = sqrt(d2) written fp16 into a
    [128, 34816] d_all buffer. Diagonal-crossing 128x128 squares are
    pre-masked on DVE in PSUM (d2 := max(d2,0) + 1e9*tril) so masked pairs
    get d ~ 3.16e4 and contribute exactly 0 to every sum; below-diagonal
    rectangles are memset to the same sentinel post-sqrt.
  - Threshold sums split by regime (classified from host dmin/dmax like the
    baseline):
      sat:    S = cnt (host).
      tails (r <= dmin-0.4): S = e^{K(r-5)} * T1 with T1 from one fused-accum
              ACT exp pass.
      sigmoid mids (r < dmin+0.85): one exact fused-accum ACT sigmoid each.
      trap+step mids (r < dmin+2.25): 3 DVE passes — hinge sums
              M(c)=sum(min(d,c)) at c = r +- 0.48 and an exact count
              F(r)=sum(d<r); host mixes the implied PWL window count
              C = P - [M(hi)-M(lo)]/(hi-lo) with F via moment-matched
              weights (kills the 1st AND 3rd moments of the
              estimator-vs-sigmoid kernel difference).
      window mids: 2 DVE hinge passes at r +- pi/K (1st-moment-matched
              smoothed count).
      count mids (near dmax): 1 DVE is_lt pass (plain count; error
              ~ rho'*pi^2/6K^2, negligible there).
    All DVE passes are tensor_scalar(op0, add-reduce accum) on fp16 SBUF
    operands -> 4x DVE perf mode, ~9us per full pass (vs 29us ACT pass).
  - Accumulators land in per-(pass,group) fp32 columns; host reduces in
    fp64, assembles the 16 sums, tiny lstsq.

Work is cut into 17 4-tile granules (two 4-bank PSUM groups, ping-pong) and 5
chunk-pair groups for threshold passes, so DVE accumulation starts ~6us in.
"""

import os
import numpy as np

import concourse.bass as bass
import concourse.mybir as mybir
from concourse.bass_utils import run_bass_kernel_spmd

N = 8192
D = 32
NC = 8
KA = 36
KSHARP = 10.0
BLK = 1024
CHW = 512
NCHUNK = 9
EXP_SHIFT = 5.0
MASK_BIG = 1e9
MASK_D = 31616.0
TAIL_MARGIN = 0.40
SAT_Z = 18.0
SIG_MARGIN = 0.85       # mids below dmin+this: exact ACT sigmoid
TS_MARGIN = 2.25        # mids below dmin+this: 3-pass trap+step
CNT_MARGIN = 2.50       # mids above dmax-this: 1-pass is_lt count
W0 = np.pi / KSHARP
WTS = float(np.pi * np.sqrt(7.0 / 3.0) / KSHARP)

GRAN_OF_CHUNK = [1] + [2] * 8
GROUPS = [(0, 1), (2, 3), (4, 5), (6, 7), (8,)]


def _chunk_tiles(k):
    return range(4) if k == 0 else range(8)


def _chunk_width(k):
    return len(_chunk_tiles(k)) * CHW


def _chunk_off(k):
    return sum(_chunk_width(j) for j in range(k))


def _group_layout(g):
    chunks = GROUPS[g]
    off = _chunk_off(chunks[0])
    width = sum(_chunk_width(k) for k in chunks)
    return off, width


def _granules():
    out = []
    for k in range(NCHUNK):
        for h in range(GRAN_OF_CHUNK[k]):
            out.append((k, h, _chunk_off(k) + 2048 * h))
    return out


GRANS = _granules()
W_TOTAL = _chunk_off(NCHUNK)           # 34816
GROUP_GRAN_PREFIX = [3, 7, 11, 15, 17]

_cache = {}
last_results = None
last_in_maps = None
_last_key = None


def _chunk_assignment():
    offdiag = []
    for i in range(NC):
        for j in range(i + 1, NC):
            for h in range(2):
                offdiag.append((i, 2 * j + h))
    assert len(offdiag) == 56
    return [[(c, 2 * c), (c, 2 * c + 1)] + offdiag[7 * c:7 * c + 7] for c in range(NC)]


def _build_program(n_sig, passes, repeat=1):
    """n_sig: exact ACT sigmoid mids (bias cols 1..n_sig hold K*r_m).
    passes: tuple of ("min"|"lt", c) DVE accumulation passes.
    Accumulator columns per group: [T1, sig..., passes...]."""
    n_ps = len(passes)
    n_cols = 1 + n_sig + n_ps
    outc = n_cols * len(GROUPS)
    nbias = 1 + n_sig
    f32 = mybir.dt.float32
    f16 = mybir.dt.float16
    bf16 = mybir.dt.bfloat16
    AF = mybir.ActivationFunctionType
    ALU = mybir.AluOpType

    nc = bass.Bass("TRN2", target_bir_lowering=False, debug=False)
    rows_d = nc.dram_tensor("rows", [KA, NCHUNK * BLK], f16, kind="ExternalInput").ap()
    cols_d = nc.dram_tensor("cols", [KA, NCHUNK * CHW], f16, kind="ExternalInput").ap()
    mask_d = nc.dram_tensor("mask", [128, 128], f32, kind="ExternalInput").ap()
    bias_d = nc.dram_tensor("bias", [128, nbias], f32, kind="ExternalInput").ap()
    out_d = nc.dram_tensor("out", [128, outc], f32, kind="ExternalOutput").ap()

    NGR = len(GRANS)
    ALL_DONE = 8 * 16

    from contextlib import ExitStack
    with ExitStack() as ctx:
        rows = ctx.enter_context(nc.sbuf_tensor("rows_sb", [KA, NCHUNK * BLK], f16)).ap()
        cols = ctx.enter_context(nc.sbuf_tensor("cols_sb", [KA, NCHUNK * CHW], f16)).ap()
        mask = ctx.enter_context(nc.sbuf_tensor("mask_sb", [128, 128], f32)).ap()
        bias = ctx.enter_context(nc.sbuf_tensor("bias_sb", [128, nbias], f32)).ap()
        dall = ctx.enter_context(nc.sbuf_tensor("d_sb", [128, W_TOTAL], f16)).ap()
        scra = ctx.enter_context(nc.sbuf_tensor("scra_sb", [128, 8192], bf16)).ap()
        scrd = ctx.enter_context(nc.sbuf_tensor("scrd_sb", [128, 8192], f16)).ap()
        acc = ctx.enter_context(nc.sbuf_tensor("acc_sb", [128, outc], f32)).ap()
        psum = [ctx.enter_context(nc.psum_tensor(f"ps{i}", [128, 2048], f32)).ap()
                for i in range(2)]
        dma_sem = ctx.enter_context(nc.semaphore("dma_sem"))
        pe_sem = ctx.enter_context(nc.semaphore("pe_sem"))
        fix_sem = ctx.enter_context(nc.semaphore("fix_sem"))
        sqrt_sem = ctx.enter_context(nc.semaphore("sqrt_sem"))
        mem_sem = ctx.enter_context(nc.semaphore("mem_sem"))
        done_sem = ctx.enter_context(nc.semaphore("done_sem"))
        block = ctx.enter_context(nc.Block())

        @block.gpsimd
        def _(g):
            RQ = NCHUNK * BLK // 4
            for q in range(4):
                g.dma_start(out=rows[:, RQ * q:RQ * (q + 1)],
                            in_=rows_d[:, RQ * q:RQ * (q + 1)]).then_inc(dma_sem, 16)
            CQ = NCHUNK * CHW // 2
            for q in range(2):
                g.dma_start(out=cols[:, CQ * q:CQ * (q + 1)],
                            in_=cols_d[:, CQ * q:CQ * (q + 1)]).then_inc(dma_sem, 16)
            g.dma_start(out=mask, in_=mask_d).then_inc(dma_sem, 16)
            g.dma_start(out=bias, in_=bias_d).then_inc(dma_sem, 16)
            g.wait_ge(done_sem, 2)
            g.dma_start(out=out_d, in_=acc).then_inc(dma_sem, 16)

        @block.tensor
        def _(t):
            t.wait_ge(dma_sem, ALL_DONE)
            for it in range(repeat):
                for gi, (k, h, _off) in enumerate(GRANS):
                    G = it * NGR + gi
                    if G >= 2:
                        t.wait_ge(sqrt_sem, G - 1)
                    mm = None
                    for j in range(4):
                        ti = 4 * h + j
                        mm = t.matmul(
                            psum[G % 2][:, CHW * j:CHW * (j + 1)],
                            lhsT=rows[:, BLK * k + 128 * ti:BLK * k + 128 * (ti + 1)],
                            rhs=cols[:, CHW * k:CHW * (k + 1)],
                            start=True, stop=True,
                        )
                    mm.then_inc(pe_sem, 1)

        @block.vector
        def _(v):
            for it in range(repeat):
                if it == 0:
                    v.wait_ge(dma_sem, ALL_DONE)
                # diag-crossing fixes in PSUM (granules 0 and 2)
                for fi, gi in enumerate((0, 2)):
                    v.wait_ge(pe_sem, it * NGR + gi + 1)
                    op = None
                    for j in range(4):
                        sq0 = CHW * j + 128 * j
                        op = v.scalar_tensor_tensor(
                            psum[gi % 2][:, sq0:sq0 + 128],
                            psum[gi % 2][:, sq0:sq0 + 128], 0.0, mask,
                            ALU.max, ALU.add)
                    op.then_inc(fix_sem, 1)
                # memsets of below-diagonal rectangles (post-sqrt)
                v.wait_ge(sqrt_sem, it * NGR + 1)
                for j in (1, 2, 3):
                    op = v.memset(dall[:, CHW * j:CHW * j + 128 * j], MASK_D)
                v.wait_ge(sqrt_sem, it * NGR + 3)
                for j in (1, 2, 3):
                    op = v.memset(dall[:, 4096 + CHW * j:4096 + CHW * j + 128 * j],
                                  MASK_D)
                op.then_inc(mem_sem, 1)
                # accumulation passes per group: accum = sum(op0(d, c))
                for g in range(len(GROUPS)):
                    off, W = _group_layout(g)
                    v.wait_ge(sqrt_sem, it * NGR + GROUP_GRAN_PREFIX[g])
                    if g == 0:
                        v.wait_ge(mem_sem, it + 1)
                    col = g * n_cols + 1 + n_sig
                    op = None
                    for ci, (kind, c) in enumerate(passes):
                        alu = ALU.min if kind == "min" else ALU.is_lt
                        op = v.tensor_scalar(
                            scrd[:, :W], dall[:, off:off + W], float(c), None,
                            alu, ALU.add,
                            accum_out=acc[:, col + ci:col + ci + 1])
                    if g == len(GROUPS) - 1 and it == repeat - 1:
                        op.then_inc(done_sem, 1)

        @block.scalar
        def _(sc):
            sc.wait_ge(dma_sem, ALL_DONE)
            for it in range(repeat):
                for gi, (k, h, off) in enumerate(GRANS):
                    G = it * NGR + gi
                    sc.wait_ge(pe_sem, G + 1)
                    if gi == 0:
                        sc.wait_ge(fix_sem, 2 * it + 1)
                    elif gi == 2:
                        sc.wait_ge(fix_sem, 2 * it + 2)
                    sc.activation(dall[:, off:off + 2048], psum[G % 2],
                                  AF.Sqrt).then_inc(sqrt_sem, 1)
                for g in range(len(GROUPS)):
                    off, W = _group_layout(g)
                    if g == 0:
                        sc.wait_ge(mem_sem, it + 1)
                    col = g * n_cols
                    sc.activation(scra[:, :W], dall[:, off:off + W], AF.Exp,
                                  scale=-KSHARP, bias=bias[:, 0:1],
                                  accum_out=acc[:, col:col + 1])
                op = None
                for i in range(n_sig):
                    for g in range(len(GROUPS)):
                        off, W = _group_layout(g)
                        col = g * n_cols
                        op = sc.activation(scra[:, :W], dall[:, off:off + W],
                                           AF.Sigmoid, scale=-KSHARP,
                                           bias=bias[:, 1 + i:2 + i],
                                           accum_out=acc[:, col + 1 + i:col + 2 + i])
                if it == repeat - 1:
                    (op if op is not None else sc.activation(
                        scra[:, :1], dall[:, :1], AF.Exp, scale=-KSHARP,
                        bias=bias[:, 0:1])).then_inc(done_sem, 1)
    return nc


def _dist_extremes(pts):
    sq = np.einsum("ij,ij->i", pts, pts)
    dmin, dmax = np.inf, 0.0
    B = 1024
    for i0 in range(0, N, B):
        g = pts[i0:i0 + B] @ pts.T
        d2b = sq[i0:i0 + B, None] + sq[None, :] - 2.0 * g
        for r in range(d2b.shape[0]):
            d2b[r, i0 + r] = np.inf
        dmin = min(dmin, float(np.sqrt(max(d2b.min(), 0.0))))
        for r in range(d2b.shape[0]):
            d2b[r, i0 + r] = 0.0
        dmax = max(dmax, float(np.sqrt(max(d2b.max(), 0.0))))
    return dmin, dmax


def _win_moments(r, lo, hi):
    xs = np.linspace(r - 2.5, r + 2.5, 200001)
    phi = np.clip((hi - xs) / (hi - lo), 0.0, 1.0)
    diff = phi - (xs < r)
    u = xs - r
    return (np.trapezoid(diff, xs), np.trapezoid(diff * u, xs),
            np.trapezoid(diff * u ** 3, xs))


def _sig_mu1():
    xs = np.linspace(-3.0, 3.0, 300001)
    z = -KSHARP * xs
    zc = np.minimum(z, 0)
    sig = np.where(z > 0, 1 / (1 + np.exp(-z)), np.exp(zc) / (1 + np.exp(zc)))
    diff = sig - (xs < 0)
    return np.trapezoid(diff * xs, xs)


def _plan(rv, dmin, dmax):
    """classify thresholds; build the DVE pass list and assembly recipes"""
    nr = len(rv)
    tail = [m for m in range(nr) if rv[m] <= dmin - TAIL_MARGIN]
    sat = [m for m in range(nr) if KSHARP * (rv[m] - dmax) >= SAT_Z]
    mids = [m for m in range(nr) if m not in tail and m not in sat]
    sig_mids = [m for m in mids if rv[m] < dmin + SIG_MARGIN]
    ts_mids = [m for m in mids if m not in sig_mids and rv[m] < dmin + TS_MARGIN]
    rest = [m for m in mids if m not in sig_mids and m not in ts_mids]
    cnt_mids = [m for m in rest if rv[m] > dmax - CNT_MARGIN]
    win_mids = [m for m in rest if m not in cnt_mids]

    s_mu1 = _sig_mu1()
    passes = []       # ("min"|"lt", c)
    recipe = {}       # m -> ("ts", iM1, iM2, iF, lo, hi, alpha) | ("win", ...) | ("cnt", iF)
    for m in ts_mids:
        r = rv[m]
        lo = float(np.float16(r - WTS)); hi = float(np.float16(r + WTS))
        _a0, a1, _a3 = _win_moments(r, lo, hi)
        alpha = s_mu1 / a1
        i0 = len(passes)
        passes += [("min", lo), ("min", hi), ("lt", float(np.float32(r)))]
        recipe[m] = ("ts", i0, i0 + 1, i0 + 2, lo, hi, alpha)
    for m in win_mids:
        r = rv[m]
        lo = float(np.float16(r - W0)); hi = float(np.float16(r + W0))
        i0 = len(passes)
        passes += [("min", lo), ("min", hi)]
        recipe[m] = ("win", i0, i0 + 1, lo, hi)
    for m in cnt_mids:
        i0 = len(passes)
        passes.append(("lt", float(np.float32(rv[m]))))
        recipe[m] = ("cnt", i0)
    return tail, sat, sig_mids, recipe, tuple(passes)


def kernel(points, r_values):
    global last_results, last_in_maps, _last_key
    points = np.ascontiguousarray(np.asarray(points, dtype=np.float32))
    r_values = np.asarray(r_values, dtype=np.float32)
    assert points.shape == (N, D) and r_values.shape == (16,)
    rv = r_values.astype(np.float64)
    nr = len(rv)

    dmin, dmax = _dist_extremes(points)
    tail, sat, sig_mids, recipe, passes = _plan(rv, dmin, dmax)
    n_sig = len(sig_mids)

    key = (n_sig, passes)
    if key not in _cache:
        _cache[key] = _build_program(n_sig, passes)
    nc = _cache[key]
    _last_key = key

    # augmented fp16 operands with split |x|^2
    p64 = points.astype(np.float64)
    sq = np.einsum("ij,ij->i", p64, p64)
    hi = np.float16(sq).astype(np.float64)
    lo = np.float16(sq - hi).astype(np.float64)
    ones = np.ones(N)
    A = np.concatenate([(-2.0 * p64).T, ones[None, :], ones[None, :],
                        hi[None, :], lo[None, :]], axis=0)
    B = np.concatenate([p64.T, hi[None, :], lo[None, :],
                        ones[None, :], ones[None, :]], axis=0)
    A16 = A.astype(np.float16)
    B16 = B.astype(np.float16)

    assign = _chunk_assignment()
    maskarr = MASK_BIG * np.tril(np.ones((128, 128), dtype=np.float32))
    nbias = 1 + n_sig
    biasarr = np.zeros((128, nbias), dtype=np.float32)
    biasarr[:, 0] = KSHARP * EXP_SHIFT
    for i, m in enumerate(sig_mids):
        biasarr[:, 1 + i] = KSHARP * rv[m]
    in_maps = []
    for c in range(NC):
        rowsb = np.empty((KA, NCHUNK * BLK), dtype=np.float16)
        colsb = np.empty((KA, NCHUNK * CHW), dtype=np.float16)
        for k, (rb, ch) in enumerate(assign[c]):
            rowsb[:, k * BLK:(k + 1) * BLK] = A16[:, rb * BLK:(rb + 1) * BLK]
            colsb[:, k * CHW:(k + 1) * CHW] = B16[:, ch * CHW:(ch + 1) * CHW]
        in_maps.append({"rows": rowsb, "cols": colsb, "mask": maskarr,
                        "bias": biasarr})
    last_in_maps = in_maps

    trace = bool(os.environ.get("CDL_TRACE"))
    res = run_bass_kernel_spmd(nc, in_maps, core_ids=list(range(NC)), trace=trace)
    last_results = res

    n_ps = len(passes)
    n_cols = 1 + n_sig + n_ps
    totals = np.zeros(n_cols, dtype=np.float64)
    for c in range(NC):
        accm = res.results[c]["out"].astype(np.float64)
        for g in range(len(GROUPS)):
            totals += accm[:, g * n_cols:(g + 1) * n_cols].sum(axis=0)

    cnt = N * (N - 1) / 2.0
    Ne = NC * W_TOTAL * 128
    T1 = totals[0]
    pv = totals[1 + n_sig:]
    S = np.zeros(nr, dtype=np.float64)
    for m in tail:
        S[m] = np.exp(KSHARP * (rv[m] - EXP_SHIFT)) * T1
    for i, m in enumerate(sig_mids):
        S[m] = totals[1 + i]
    for m, rec in recipe.items():
        if rec[0] == "ts":
            _t, i1, i2, iF, lo_w, hi_w, alpha = rec
            C = Ne - (pv[i2] - pv[i1]) / (hi_w - lo_w)
            S[m] = alpha * C + (1 - alpha) * pv[iF]
        elif rec[0] == "win":
            _t, i1, i2, lo_w, hi_w = rec
            S[m] = Ne - (pv[i2] - pv[i1]) / (hi_w - lo_w)
        else:
            S[m] = pv[rec[1]]
    for m in sat:
        S[m] = cnt

    corr = S / cnt
    logr = np.log(rv)
    logc = np.log(corr)
    Amat = np.stack([logr, np.ones_like(logr)], axis=1)
    sol = np.linalg.solve(Amat.T @ Amat, Amat.T @ logc)
    return np.asarray(-sol[0], dtype=np.float32)


def build_repeat(repeat):
    n_sig, passes = _last_key
    return _build_program(n_sig, passes, repeat=repeat)
